# revision 3
# baseline (speedup 1.0000x reference)
"""Multi-head self-attention (B=4, S=2048, D=1024, H=16, causal) on 8 TRN2 cores.

Sharding: core = (batch b, head-group g) with b = core//2, g = core%2.
Each core computes Q/K/V projections for its batch restricted to its 8 heads
(column-parallel), causal flash attention for those heads, and a row-parallel
partial of the output projection. Host sums the two partials per batch and
adds the bias terms. Zero collectives; every core runs the identical program
on different data.

On-device layout is feature-on-partition ("transposed"): qT/kT [e, t] so the
scores matmul produces sT [k, q] tiles directly (no PE transposes). Softmax
skips max-subtraction (scores ~ N(0,1) after the 1/sqrt(dh) fold into Wq, so
fp32 exp is safe). The softmax denominator comes for free as a 65th "ones"
row in the AV matmul. All matmuls run in float32r (full fp32 storage,
bf16-rate PE, ~13-bit mantissa).
"""

import numpy as np

B = 4
S = 2048
D = 1024
H = 16
DH = 64
HG = 8            # heads per core
E = HG * DH       # 512 features per head-group
P = 128
NCORES = 8

DC = D // P       # 8 d-chunks
EC = E // P       # 4 e-chunks per group
TC4 = S // 512    # 4 token 512-chunks
TT = S // P       # 16 token 128-tiles
QB = S // 512     # 4 query blocks of 512
NEG = -1.0e9

_CACHE = {}


def _build_nc():
    import concourse.mybir as mybir
    from concourse import bacc
    from concourse.tile import TileContext

    f32 = mybir.dt.float32
    f32r = mybir.dt.float32r

    nc = bacc.Bacc("TRN2", target_bir_lowering=False, name="mhsa")
    xT = nc.dram_tensor("xT", [D, S], f32r, kind="ExternalInput")
    wq = nc.dram_tensor("wq", [D, E], f32r, kind="ExternalInput")
    wk = nc.dram_tensor("wk", [D, E], f32r, kind="ExternalInput")
    wv = nc.dram_tensor("wv", [D, E], f32r, kind="ExternalInput")
    wo = nc.dram_tensor("wo", [E, D], f32r, kind="ExternalInput")
    bq = nc.dram_tensor("bq", [P, EC], f32, kind="ExternalInput")
    bk = nc.dram_tensor("bk", [P, EC], f32, kind="ExternalInput")
    cm = nc.dram_tensor("cm", [P, 4, 512], f32, kind="ExternalInput")
    outp = nc.dram_tensor("outp", [S, D], f32, kind="ExternalOutput")

    with TileContext(nc) as tc:
        with tc.tile_pool(name="persist", bufs=1) as persist, \
             tc.tile_pool(name="dram", bufs=2, space="DRAM") as dram_pool, \
             tc.tile_pool(name="ps_proj", bufs=2, space="PSUM") as ps_proj, \
             tc.tile_pool(name="ps_s", bufs=4, space="PSUM") as ps_s_pool, \
             tc.tile_pool(name="ps_av", bufs=2, space="PSUM") as ps_av_pool:

            qT_all = persist.tile([P, EC, S], f32r)      # 4 MB
            kT_all = persist.tile([P, EC, S], f32r)      # 4 MB
            v_aug = persist.tile([P, TT, HG, DH + 1], f32r)  # ~4.3 MB
            cT_all = persist.tile([P, EC, S], f32r)      # 4 MB
            cm_sb = persist.tile([P, 4, 512], f32)       # 1 MB
            bq_sb = persist.tile([P, EC], f32)
            bk_sb = persist.tile([P, EC], f32)
            nc.sync.dma_start(cm_sb, cm.ap())
            nc.sync.dma_start(bq_sb, bq.ap())
            nc.sync.dma_start(bk_sb, bk.ap())

            # ones columns of v_aug
            ones_f32 = persist.tile([P, TT, HG], f32)
            nc.vector.memset(ones_f32, 1.0)
            nc.vector.tensor_copy(v_aug[:, :, :, DH], ones_f32)

            # ---------------- Phase 1: projections ----------------
            TCH = 256                     # token chunk (f32r full rate at N>=256)
            NCH = S // TCH
            with tc.tile_pool(name="wpool", bufs=1) as wpool, \
                 tc.tile_pool(name="xpool", bufs=2) as xpool:
                # Q and K in transposed layout qT[e, t]
                for w_dram, b_sb, dst in ((wq, bq_sb, qT_all),
                                          (wk, bk_sb, kT_all)):
                    w_sb = wpool.tile([P, DC, E], f32r, tag="w")
                    nc.sync.dma_start(
                        w_sb, w_dram.rearrange("(dc p) e -> p dc e", p=P))
                    for t4 in range(NCH):
                        ts_ = slice(t4 * TCH, (t4 + 1) * TCH)
                        xt = xpool.tile([P, DC, TCH], f32r, tag="xt")
                        nc.sync.dma_start(
                            xt, xT[:, ts_].rearrange("(dc p) t -> p dc t", p=P))
                        for ec in range(EC):
                            ps = ps_proj.tile([P, TCH], mybir.dt.float32,
                                              tag="pp")
                            for dc in range(DC):
                                nc.tensor.matmul(
                                    ps, w_sb[:, dc, ec * P:(ec + 1) * P],
                                    xt[:, dc],
                                    start=(dc == 0), stop=(dc == DC - 1))
                            nc.scalar.activation(
                                dst[:, ec, ts_], ps,
                                mybir.ActivationFunctionType.Identity,
                                bias=b_sb[:, ec:ec + 1])

                # V in natural layout v[t, e] (+ ones col); bv folded on host
                wv_sb = wpool.tile([P, DC, E], f32r, tag="w")
                nc.sync.dma_start(
                    wv_sb, wv.rearrange("(dc p) e -> p dc e", p=P))
                for t4 in range(NCH):
                    ts_ = slice(t4 * TCH, (t4 + 1) * TCH)
                    xt = xpool.tile([P, DC, TCH], f32r, tag="xt")
                    nc.sync.dma_start(
                        xt, xT[:, ts_].rearrange("(dc p) t -> p dc t", p=P))
                    for tb in range(TCH // P):
                        ps = ps_proj.tile([P, E], mybir.dt.float32, tag="pp")
                        for dc in range(DC):
                            nc.tensor.matmul(
                                ps, xt[:, dc, tb * P:(tb + 1) * P], wv_sb[:, dc],
                                start=(dc == 0), stop=(dc == DC - 1))
                        tt = t4 * (TCH // P) + tb
                        nc.vector.tensor_copy(
                            v_aug[:, tt, :, 0:DH],
                            ps.rearrange("p (h d) -> p h d", h=HG))

            # ---------------- Phase 2: attention ----------------
            with tc.tile_pool(name="ptpool", bufs=4) as ptpool, \
                 tc.tile_pool(name="smpool", bufs=2) as smpool, \
                 tc.tile_pool(name="normpool", bufs=2) as normpool:
                for hp in range(EC):      # head pair chunk: heads 2hp, 2hp+1
                    for qb in range(QB):
                        nkt = qb * 4 + 4  # causal k-tiles for this q block
                        ps_av0 = ps_av_pool.tile([DH + 1, 512],
                                                 mybir.dt.float32, tag="av")
                        ps_av1 = ps_av_pool.tile([DH + 1, 512],
                                                 mybir.dt.float32, tag="av")
                        qs = slice(qb * 512, (qb + 1) * 512)
                        for kt in range(nkt):
                            ks = slice(kt * P, (kt + 1) * P)
                            ps_s0 = ps_s_pool.tile([P, 512], mybir.dt.float32,
                                                   tag="s")
                            ps_s1 = ps_s_pool.tile([P, 512], mybir.dt.float32,
                                                   tag="s")
                            nc.tensor.matmul(ps_s0, kT_all[0:DH, hp, ks],
                                             qT_all[0:DH, hp, qs],
                                             start=True, stop=True,
                                             tile_position=(0, 0))
                            nc.tensor.matmul(ps_s1, kT_all[DH:P, hp, ks],
                                             qT_all[DH:P, hp, qs],
                                             start=True, stop=True,
                                             tile_position=(64, 0))
                            pt0 = ptpool.tile([P, 512], f32r, tag="pt")
                            pt1 = ptpool.tile([P, 512], f32r, tag="pt")
                            if kt >= qb * 4:  # diagonal tile: causal bias
                                j = kt - qb * 4
                                for ps_sx, ptx in ((ps_s0, pt0), (ps_s1, pt1)):
                                    sm = smpool.tile([P, 512], f32, tag="sm")
                                    nc.vector.tensor_tensor(
                                        sm, ps_sx, cm_sb[:, j],
                                        mybir.AluOpType.add)
                                    nc.scalar.activation(
                                        ptx, sm,
                                        mybir.ActivationFunctionType.Exp)
                            else:
                                nc.scalar.activation(
                                    pt0, ps_s0,
                                    mybir.ActivationFunctionType.Exp)
                                nc.scalar.activation(
                                    pt1, ps_s1,
                                    mybir.ActivationFunctionType.Exp)
                            nc.tensor.matmul(
                                ps_av0, v_aug[:, kt, 2 * hp], pt0,
                                start=(kt == 0), stop=(kt == nkt - 1))
                            nc.tensor.matmul(
                                ps_av1, v_aug[:, kt, 2 * hp + 1], pt1,
                                start=(kt == 0), stop=(kt == nkt - 1))
                        # normalize: c = av[0:64] * (1/av[64]) bcast over rows
                        for idx, ps_av in ((0, ps_av0), (1, ps_av1)):
                            recip = normpool.tile([1, 512], f32, tag="recip")
                            nc.vector.reciprocal(recip, ps_av[DH:DH + 1])
                            r_dram = dram_pool.tile([1, 512], f32, tag="rd")
                            nc.sync.dma_start(r_dram, recip)
                            r_rep = normpool.tile([DH, 512], f32, tag="rrep")
                            nc.sync.dma_start(
                                r_rep, r_dram.to_broadcast([DH, 512]))
                            nc.vector.tensor_tensor(
                                cT_all[idx * DH:(idx + 1) * DH, hp, qs],
                                ps_av[0:DH], r_rep, mybir.AluOpType.mult)

            # ---------------- Phase 3: partial out-projection ----------------
            with tc.tile_pool(name="wopool", bufs=1) as wopool, \
                 tc.tile_pool(name="evict", bufs=2) as evict:
                wo_sb = wopool.tile([P, EC, D], f32r, tag="wo")
                nc.sync.dma_start(
                    wo_sb, wo.rearrange("(dc p) e -> p dc e", p=P))
                for tb in range(TT):
                    for eb in range(2):
                        ps = ps_proj.tile([P, 512], mybir.dt.float32, tag="pp")
                        for dc in range(EC):
                            nc.tensor.matmul(
                                ps, cT_all[:, dc, tb * P:(tb + 1) * P],
                                wo_sb[:, dc, eb * 512:(eb + 1) * 512],
                                start=(dc == 0), stop=(dc == EC - 1))
                        o_sb = evict.tile([P, 512], f32, tag="o")
                        nc.any.tensor_copy(o_sb, ps)
                        nc.sync.dma_start(
                            outp.ap()[tb * P:(tb + 1) * P,
                                      eb * 512:(eb + 1) * 512],
                            o_sb)

    nc.finalize()
    return nc


def make_in_maps(x, Wq, bq, Wk, bk, Wv, bv, Wo, bo, mask):
    """Build the 8 per-core input dicts (host-side shard + transform)."""
    x = np.asarray(x, dtype=np.float32)
    Wq = np.asarray(Wq, dtype=np.float32)
    Wk = np.asarray(Wk, dtype=np.float32)
    Wv = np.asarray(Wv, dtype=np.float32)
    Wo = np.asarray(Wo, dtype=np.float32)
    bqf = np.asarray(bq, dtype=np.float32)
    bkf = np.asarray(bk, dtype=np.float32)
    mask = np.asarray(mask)

    scale = 1.0 / np.sqrt(np.float32(DH))
    # torch convention y = x @ W.T: feed W.T with d_in on axis 0
    WqT = np.ascontiguousarray(Wq.T) * scale        # [D, D], scale folded
    WkT = np.ascontiguousarray(Wk.T)
    WvT = np.ascontiguousarray(np.asarray(Wv, dtype=np.float32).T)
    WoT = np.ascontiguousarray(Wo.T)                # [D, D]

    # causal diag bias tiles from the mask input: tile j covers keys
    # [q0+128j, q0+128j+128) for query block [q0, q0+512); tril is
    # translation-invariant so build from q0 = S-512.
    q0 = S - 512
    m2 = mask.reshape(S, S)
    cm = np.empty((P, 4, 512), np.float32)
    for j in range(4):
        sub = m2[q0:q0 + 512, q0 + 128 * j:q0 + 128 * j + 128]  # [q, k]
        cm[:, j, :] = np.where(sub.T != 0, 0.0, NEG)

    in_maps = []
    for core in range(NCORES):
        b, g = divmod(core, 2)
        cols = slice(g * E, (g + 1) * E)
        in_maps.append({
            "xT": np.ascontiguousarray(x[b].T),          # [D, S]
            "wq": np.ascontiguousarray(WqT[:, cols]),
            "wk": np.ascontiguousarray(WkT[:, cols]),
            "wv": np.ascontiguousarray(WvT[:, cols]),
            "wo": np.ascontiguousarray(WoT[cols, :]),
            "bq": np.ascontiguousarray((bqf[cols] * scale).reshape(EC, P).T),
            "bk": np.ascontiguousarray(bkf[cols].reshape(EC, P).T),
            "cm": cm,
            "outp": np.zeros((S, D), np.float32),
        })
    for m in in_maps:
        m.pop("outp")
    return in_maps


def assemble_output(results, bv, bo, Wo):
    """Sum per-batch partials and add the bias correction."""
    bv = np.asarray(bv, dtype=np.float32)
    bo = np.asarray(bo, dtype=np.float32)
    Wo = np.asarray(Wo, dtype=np.float32)
    # context bias bv contributes bv @ Wo.T (attn rows sum to 1)
    corr = (bo + bv @ Wo.T).astype(np.float32)      # [D]
    out = np.empty((B, S, D), np.float32)
    for b in range(B):
        out[b] = results[2 * b]["outp"] + results[2 * b + 1]["outp"] + corr
    return out


def kernel(x, Wq, bq, Wk, bk, Wv, bv, Wo, bo, mask):
    from concourse.bass_utils import run_bass_kernel_spmd

    if "nc" not in _CACHE:
        _CACHE["nc"] = _build_nc()
    nc = _CACHE["nc"]
    in_maps = make_in_maps(x, Wq, bq, Wk, bk, Wv, bv, Wo, bo, mask)
    res = run_bass_kernel_spmd(nc, in_maps, core_ids=list(range(NCORES)))
    return assemble_output(res.results, bv, bo, Wo)
